# revision 25
# baseline (speedup 1.0000x reference)
"""Multi-head attention (B=2, L=2048, D=1024, H=16) on 8 NeuronCores.

Sharding: data-parallel over batch (2) x tensor-parallel over heads (4 heads
per core).  Each core computes, for its batch b and its 4 heads:
  QT = (Wq_slice*DH^-.5).T @ q[b].T        [256, L]   (dh on partitions)
  KT = Wk_slice.T @ k[b].T                 [256, L]
  V  = v[b] @ Wv_slice                     [L, 256]   (l on partitions)
  per q-tile (128 rows) and head h:
    scores = QT_h.T @ KT_h                 [128, L]   (K=64 matmuls, PE)
    probs  = exp(scores)  (fp16, ACT, accum_out -> row sums -> dinv, DVE)
    covacc += probs_h * dinv_h             (DVE scalar_tensor_tensor, fp16)
    probsT = PE-transpose(probs); pv_h += probsT.T @ V_h   [128 q, 64]
    attn_h = pv_h * dinv_h (DVE); attnT = PE-transpose(attn)
    out_partial = attnT.T @ Wo_rows        [128, 1024] -> DRAM (fp32)
Host sums the 4 partial outs per batch and averages the coverage partials
(the /H mean over heads happens on the host).
"""

import sys
from contextlib import ExitStack

import numpy as np

sys.path.insert(0, "/opt/trn_rl_repo")

import concourse.bass as bass  # noqa: E402
import concourse.tile as tile  # noqa: E402
from concourse import bacc, mybir  # noqa: E402
from concourse.bass_utils import run_bass_kernel_spmd  # noqa: E402

B, L, D, H = 2, 2048, 1024, 16
DH = D // H          # 64
NCORES = 8
CPB = NCORES // B    # cores per batch (tensor-parallel width) = 4
HPC = H // CPB       # heads per core = 4
DHC = HPC * DH       # head dims per core = 256
P = 128

FP = mybir.dt.float32
HALF = mybir.dt.float16
NP_HALF = np.float16
ACT_COPY = mybir.ActivationFunctionType.Copy


def build_nc(l=L, trace_sim=False, loop_n=1):
    """Build the per-core Bass program (same program on all 8 cores).

    loop_n > 1 wraps the whole body in an on-device For_i loop — used only
    by the timing rig to amortize host/tunnel dispatch overhead.
    """
    import contextlib

    nc = bacc.Bacc("TRN2", target_bir_lowering=False)

    qT = nc.declare_dram_parameter("qT", [D, l], HALF, isOutput=False).ap()
    kT = nc.declare_dram_parameter("kT", [D, l], HALF, isOutput=False).ap()
    vT = nc.declare_dram_parameter("vT", [D, l], HALF, isOutput=False).ap()
    wq = nc.declare_dram_parameter("wq", [D, DHC], HALF, isOutput=False).ap()
    wk = nc.declare_dram_parameter("wk", [D, DHC], HALF, isOutput=False).ap()
    wv = nc.declare_dram_parameter("wv", [D, DHC], HALF, isOutput=False).ap()
    wo = nc.declare_dram_parameter("wo", [DHC, D], HALF, isOutput=False).ap()
    out_p = nc.declare_dram_parameter("out_p", [l, D], FP, isOutput=True).ap()
    cov_p = nc.declare_dram_parameter("cov_p", [l, l], HALF, isOutput=True).ap()

    DC = D // P          # 8 contraction chunks of 128
    LT = l // P          # q tiles
    EK = min(1024, l)    # exp chunk width
    KC2 = l // EK        # exp chunks
    EJ = EK // 512       # 512-wide matmuls per exp chunk
    KT16 = l // P        # 128-wide k chunks
    KG = l // 512        # 512-wide groups for transposes

    with tile.TileContext(nc, trace_sim=trace_sim) as tc, ExitStack() as ctx:
        singles = ctx.enter_context(tc.tile_pool(name="singles", bufs=1))
        inpool = ctx.enter_context(tc.tile_pool(name="inpool", bufs=1))
        persist = ctx.enter_context(tc.tile_pool(name="persist", bufs=1))
        probs_pool = ctx.enter_context(tc.tile_pool(name="probs", bufs=3))
        stat_pool = ctx.enter_context(tc.tile_pool(name="stats", bufs=4))
        pT_pool = ctx.enter_context(tc.tile_pool(name="pT", bufs=4))
        attn_pool = ctx.enter_context(tc.tile_pool(name="attn", bufs=2))
        attnT_pool = ctx.enter_context(tc.tile_pool(name="attnT", bufs=2))
        cov_out_pool = ctx.enter_context(tc.tile_pool(name="cov_out", bufs=3))
        covtmp_pool = ctx.enter_context(tc.tile_pool(name="covtmp", bufs=2))
        out_out_pool = ctx.enter_context(tc.tile_pool(name="out_out", bufs=3))

        # PSUM budget (8 banks): sc 2x2 + T 2x1 + pv 2x1 = 8
        ps_sc = ctx.enter_context(tc.tile_pool(name="ps_sc", bufs=2, space="PSUM"))
        ps_T = ctx.enter_context(tc.tile_pool(name="ps_T", bufs=2, space="PSUM"))
        ps_pv = ctx.enter_context(tc.tile_pool(name="ps_pv", bufs=2, space="PSUM"))

        loop_ctx = tc.For_i(0, loop_n, 1) if loop_n > 1 else contextlib.nullcontext()
        ctx.enter_context(loop_ctx)

        # identity for PE transposes
        ident_raw = singles.tile([P, P], HALF)
        nc.gpsimd.memset(ident_raw, 0.0)
        nc.gpsimd.affine_select(
            out=ident_raw, in_=ident_raw, compare_op=mybir.AluOpType.not_equal,
            fill=1.0, base=0, pattern=[[-1, P]], channel_multiplier=1,
        )
        ident = singles.tile([P, P], HALF)
        nc.vector.tensor_copy(out=ident, in_=ident_raw)

        # ---- load inputs ----
        wq_sb = singles.tile([P, DC, DHC], HALF)
        wk_sb = singles.tile([P, DC, DHC], HALF)
        wv_sb = singles.tile([P, DC, DHC], HALF)
        wo_sb = singles.tile([P, DHC // P, D], HALF)
        for c in range(DC):
            nc.sync.dma_start(out=wq_sb[:, c, :], in_=wq[c * P:(c + 1) * P, :])
            nc.sync.dma_start(out=wk_sb[:, c, :], in_=wk[c * P:(c + 1) * P, :])
            nc.sync.dma_start(out=wv_sb[:, c, :], in_=wv[c * P:(c + 1) * P, :])
        for t in range(DHC // P):
            nc.sync.dma_start(out=wo_sb[:, t, :], in_=wo[t * P:(t + 1) * P, :])

        # ---- projections ----
        # inputs are streamed one at a time through a shared 2-deep slot
        QT_sb = persist.tile([P, 2, l], HALF)
        KT_sb = persist.tile([P, 2, l], HALF)
        V_sb = persist.tile([P, LT, DHC], HALF)

        for which, src, wsb in (("q", qT, wq_sb), ("k", kT, wk_sb),
                                ("v", vT, wv_sb)):
            x_sb = inpool.tile([P, DC, l], HALF, tag="inx", bufs=2,
                               name=f"in_{which}")
            for c in range(DC):
                nc.sync.dma_start(out=x_sb[:, c, :], in_=src[c * P:(c + 1) * P, :])
            if which in ("q", "k"):
                dst = QT_sb if which == "q" else KT_sb
                for t in range(2):
                    for lc in range(l // 512):
                        pq = ps_sc.tile([P, 1024], FP, tag="sc")
                        for c in range(DC):
                            nc.tensor.matmul(
                                pq[:, 0:512], lhsT=wsb[:, c, t * P:(t + 1) * P],
                                rhs=x_sb[:, c, lc * 512:(lc + 1) * 512],
                                start=(c == 0), stop=(c == DC - 1),
                            )
                        nc.scalar.activation(
                            out=dst[:, t, lc * 512:(lc + 1) * 512],
                            in_=pq[:, 0:512], func=ACT_COPY)
            else:
                # V: [l -> LT tiles of 128, dh 256]
                for lt in range(LT):
                    pv = ps_pv.tile([P, DHC], FP, tag="pv")
                    for c in range(DC):
                        nc.tensor.matmul(
                            pv, lhsT=x_sb[:, c, lt * P:(lt + 1) * P],
                            rhs=wsb[:, c, :],
                            start=(c == 0), stop=(c == DC - 1),
                        )
                    nc.scalar.activation(out=V_sb[:, lt, :], in_=pv,
                                         func=ACT_COPY)

        # ---- attention per q-tile ----
        for qt in range(LT):
            probs = probs_pool.tile([P, HPC, l], HALF)
            rs = stat_pool.tile([P, HPC, KC2], FP)
            dinv = stat_pool.tile([P, HPC], FP)
            covacc = cov_out_pool.tile([P, l], HALF)
            covtmp = covtmp_pool.tile([P, 3, l], HALF)
            pvac = ps_pv.tile([P, DHC], FP, tag="pv")
            attn = attn_pool.tile([P, DHC], HALF)
            for h in range(HPC):
                po = (h % 2) * DH          # partition offset of head in QT tile
                ti = h // 2                # which 128-tile of QT/KT
                for kc in range(KC2):
                    ps_s = ps_sc.tile([P, 1024], FP, tag="sc")
                    for j in range(EJ):
                        nc.tensor.matmul(
                            ps_s[:, j * 512:(j + 1) * 512],
                            lhsT=QT_sb[po:po + DH, ti, qt * P:(qt + 1) * P],
                            rhs=KT_sb[po:po + DH, ti,
                                      kc * EK + j * 512:kc * EK + (j + 1) * 512],
                            start=True, stop=True,
                        )
                    nc.scalar.activation(
                        out=probs[:, h, kc * EK:(kc + 1) * EK], in_=ps_s[:, 0:EK],
                        func=mybir.ActivationFunctionType.Exp,
                        accum_out=rs[:, h, kc:kc + 1],
                    )
                nc.vector.tensor_reduce(
                    out=dinv[:, h:h + 1], in_=rs[:, h, :],
                    axis=mybir.AxisListType.X, op=mybir.AluOpType.add,
                )
                nc.vector.reciprocal(out=dinv[:, h:h + 1], in_=dinv[:, h:h + 1])
                # normalized per-head probs (DVE tensor_scalar runs at 4x)
                tgt = covacc if h == 0 else covtmp[:, h - 1, :]
                nc.vector.tensor_scalar_mul(tgt, probs[:, h, :], dinv[:, h:h + 1])
                # transpose + PV for this head
                for kg in range(KG):
                    pt_ps = ps_T.tile([P, 512], HALF, tag="T")
                    for j in range(4):
                        nc.tensor.transpose(
                            pt_ps[:, j * P:(j + 1) * P],
                            probs[:, h, kg * 512 + j * P:kg * 512 + (j + 1) * P],
                            ident)
                    pT = pT_pool.tile([P, 512], HALF)
                    nc.vector.tensor_copy(out=pT, in_=pt_ps)
                    for j in range(4):
                        kt = kg * 4 + j
                        nc.tensor.matmul(
                            pvac[:, h * DH:(h + 1) * DH],
                            lhsT=pT[:, j * P:(j + 1) * P],
                            rhs=V_sb[:, kt, h * DH:(h + 1) * DH],
                            start=(kt == 0), stop=(kt == KT16 - 1),
                        )
                nc.vector.tensor_scalar_mul(
                    attn[:, h * DH:(h + 1) * DH],
                    pvac[:, h * DH:(h + 1) * DH], dinv[:, h:h + 1])
            # sum the 4 normalized heads: 2 adds on gpsimd (otherwise idle),
            # final add on DVE
            nc.gpsimd.tensor_tensor(
                out=covtmp[:, 1, :], in0=covtmp[:, 1, :], in1=covtmp[:, 2, :],
                op=mybir.AluOpType.add)
            nc.gpsimd.tensor_tensor(
                out=covacc, in0=covacc, in1=covtmp[:, 0, :],
                op=mybir.AluOpType.add)
            nc.vector.tensor_tensor(
                out=covacc, in0=covacc, in1=covtmp[:, 1, :],
                op=mybir.AluOpType.add)
            nc.sync.dma_start(out=cov_p[qt * P:(qt + 1) * P, :], in_=covacc)

            # attnT tiles for the Wo matmul
            attnT = attnT_pool.tile([P, 2, P], HALF)
            at_ps = ps_T.tile([P, 512], HALF, tag="T")
            for t in range(2):
                nc.tensor.transpose(
                    at_ps[:, t * P:(t + 1) * P], attn[:, t * P:(t + 1) * P], ident)
            nc.vector.tensor_copy(out=attnT, in_=at_ps[:, 0:2 * P])

            # out_partial[qt] = attnT.T @ Wo_rows  ([128, D=1024])
            out_sb = out_out_pool.tile([P, D], FP)
            po_ps = ps_sc.tile([P, 1024], FP, tag="sc")
            for j in range(2):
                for t in range(2):
                    nc.tensor.matmul(
                        po_ps[:, j * 512:(j + 1) * 512], lhsT=attnT[:, t, :],
                        rhs=wo_sb[:, t, j * 512:(j + 1) * 512],
                        start=(t == 0), stop=(t == 1),
                    )
            nc.scalar.activation(out=out_sb, in_=po_ps, func=ACT_COPY)
            nc.sync.dma_start(out=out_p[qt * P:(qt + 1) * P, :], in_=out_sb)

    nc.compile()
    return nc


_NC_CACHE = {}
_LAST_RESULTS = None


def _numpy_fallback(query, key, value, mask, Wq, Wk, Wv, Wo):
    out = np.zeros((B, L, D), np.float32)
    cov = np.zeros((B, L, L), np.float32)
    for b in range(B):
        Q = (query[b] @ Wq) * DH ** -0.5
        K = key[b] @ Wk
        V = value[b] @ Wv
        for h in range(H):
            sl = slice(h * DH, (h + 1) * DH)
            s = Q[:, sl] @ K[:, sl].T
            s = np.where(mask[b], -np.inf, s)
            p = np.exp(s - s.max(-1, keepdims=True))
            p /= p.sum(-1, keepdims=True)
            cov[b] += p / H
            out[b] += (p @ V[:, sl]) @ Wo[sl, :]
    return out, cov


def kernel(query, key, value, mask, Wq, Wk, Wv, Wo):
    query = np.asarray(query, np.float32)
    key = np.asarray(key, np.float32)
    value = np.asarray(value, np.float32)
    mask = np.asarray(mask)
    Wq = np.asarray(Wq, np.float32)
    Wk = np.asarray(Wk, np.float32)
    Wv = np.asarray(Wv, np.float32)
    Wo = np.asarray(Wo, np.float32)

    if mask.any():
        # the device kernel hardcodes the no-mask case (setup_inputs fills
        # zeros); fall back to a correct host implementation otherwise
        return _numpy_fallback(query, key, value, mask, Wq, Wk, Wv, Wo)

    if "nc" not in _NC_CACHE:
        _NC_CACHE["nc"] = build_nc()
    nc = _NC_CACHE["nc"]

    qTs = [np.ascontiguousarray(query[b].T).astype(NP_HALF) for b in range(B)]
    kTs = [np.ascontiguousarray(key[b].T).astype(NP_HALF) for b in range(B)]
    vTs = [np.ascontiguousarray(value[b].T).astype(NP_HALF) for b in range(B)]
    wq_s = (Wq * DH ** -0.5).astype(NP_HALF)
    wk_s = Wk.astype(NP_HALF)
    wv_s = Wv.astype(NP_HALF)
    wo_s = Wo.astype(NP_HALF)

    in_maps = []
    for c in range(NCORES):
        b, g = divmod(c, CPB)
        hs = slice(g * DHC, (g + 1) * DHC)
        in_maps.append({
            "qT": qTs[b], "kT": kTs[b], "vT": vTs[b],
            "wq": np.ascontiguousarray(wq_s[:, hs]),
            "wk": np.ascontiguousarray(wk_s[:, hs]),
            "wv": np.ascontiguousarray(wv_s[:, hs]),
            "wo": np.ascontiguousarray(wo_s[hs, :]),
        })

    res = run_bass_kernel_spmd(nc, in_maps, core_ids=list(range(NCORES)))
    global _LAST_RESULTS
    _LAST_RESULTS = res

    out = np.zeros((B, L, D), np.float32)
    cov = np.zeros((B, L, L), np.float32)
    for c in range(NCORES):
        b = c // CPB
        out[b] += res.results[c]["out_p"]
        cov[b] += res.results[c]["cov_p"]
    cov *= 1.0 / H
    return out, cov
